# revision 2
# baseline (speedup 1.0000x reference)
"""AggrHGraphConvLayer (heterogeneous GraphConv, 6 relations) on 8 TRN2 cores.

Strategy — destination-sharded one-hot-matmul segment sum:
- Destinations of each dst-type are assigned to (core, 256-wide window pair)
  buckets, balanced by degree. Edges are routed to the core owning their dst.
- Per (relation, int16-range pass) the edges form a padded slot stream;
  every 128 slots = one matmul tile aimed at one window pair.
- On device: dma_gather pulls x[src] rows (512B) from HBM in slot order;
  ScalarE casts each tile to bf16 while applying the per-edge
  rsqrt(outdeg*indeg) scale; VectorE builds one-hot masks [128, 256] from an
  iota constant via tensor_tensor is_equal (1-port mode — never blocks the
  SWDGE gather descriptor generation); TensorE accumulates
  agg^T[f, d] += rows^T @ mask into PSUM per window pair; pairs evacuate to
  SBUF (bf16) on ScalarE, get multiplied by W per relation with PSUM
  accumulation over the dst-type's relations, and finish with
  Relu(psum/k + b/k) on ScalarE.
- Degrees arrive as host-prepared integer bincount metadata (per-slot);
  all float math (clip/rsqrt/scaling) runs on device.
"""

import sys

sys.path.insert(0, "/opt/trn_rl_repo")
import numpy as np

import concourse.bass as bass
import concourse.tile as tile
from concourse import bacc, mybir
from concourse.bass_utils import run_bass_kernel_spmd

N_CORES = 8
PAIR = 256          # dst slots per window pair (= PSUM tile free dim)
TILE = 128          # edge slots per matmul tile
CHUNK = 1024        # max idxs per dma_gather instruction (SWDGE ring limit)
HI = 32768          # int16 index range per gather pass
D = 128

# (group, n_dst, [(rel, src_table, n_src)])
GROUPS = [
    ("node", 10000, [("in", "instance_feat", 50000)]),
    ("inst", 50000, [("ni", "node_feat", 10000),
                     ("ii", "instance_feat", 50000),
                     ("si", "svc_feat", 20000)]),
    ("svc", 20000, [("sc", "svc_feat", 20000),
                    ("is", "instance_feat", 50000)]),
]

N_QUEUES = 4
STAGE_BUFS = 8
MASK_BUFS = 4
PSA_BUFS = 6
MASK_BATCH = 4
DVE_CAST_MOD = 4


def _balance_buckets(deg, n_buckets, cap):
    """Greedy balanced assignment of items (loads deg) into equal-capacity
    buckets. Returns bucket id per item."""
    import heapq
    order = np.argsort(-deg, kind="stable")
    heap = [(0, b) for b in range(n_buckets)]
    heapq.heapify(heap)
    space = np.full(n_buckets, cap, np.int64)
    out = np.empty(len(deg), np.int64)
    for d in order:
        while True:
            load, b = heapq.heappop(heap)
            if space[b] > 0:
                break
        out[d] = b
        space[b] -= 1
        if space[b] > 0:
            heapq.heappush(heap, (load + int(deg[d]), b))
    return out


def prepare(inputs):
    """Host-side integer metadata prep. Returns (in_maps, sched, meta)."""
    inp = {k: np.asarray(v) for k, v in inputs.items()}
    rels_all = {}
    for gname, n_dst, rels in GROUPS:
        for rel, tabname, n_src in rels:
            src = inp[f"e_{rel}_src"].astype(np.int64)
            dst = inp[f"e_{rel}_dst"].astype(np.int64)
            rels_all[rel] = dict(src=src, dst=dst, tab=tabname, n_src=n_src,
                                 group=gname, n_dst=n_dst,
                                 outdeg=np.bincount(src, minlength=n_src),
                                 indeg=np.bincount(dst, minlength=n_dst))

    gmeta = {}
    for gname, n_dst, rels in GROUPS:
        n_pairs = -(-((n_dst + N_CORES - 1) // N_CORES) // PAIR)
        deg = np.zeros(n_dst, np.int64)
        for rel, _, _ in rels:
            deg += rels_all[rel]["indeg"]
        bucket = _balance_buckets(deg, N_CORES * n_pairs, PAIR)
        core = bucket // n_pairs
        pair = bucket % n_pairs
        slot = np.zeros(n_dst, np.int64)
        for b in range(N_CORES * n_pairs):
            ids = np.where(bucket == b)[0]
            slot[ids] = np.arange(len(ids))
        gmeta[gname] = dict(n_pairs=n_pairs, core=core, pair=pair, slot=slot,
                            n_dst=n_dst)

    streams = []
    per_core = [dict(idx=[], dstl=[], od=[], idg=[]) for _ in range(N_CORES)]
    slot_base = 0
    for gname, n_dst, rels in GROUPS:
        gm = gmeta[gname]
        for rel, tabname, n_src in rels:
            R = rels_all[rel]
            n_pass = 2 if n_src > HI else 1
            ecore = gm["core"][R["dst"]]
            epair = gm["pair"][R["dst"]]
            for p in range(n_pass):
                if n_pass == 1:
                    sel_pass = np.ones(len(R["src"]), bool)
                else:
                    sel_pass = (R["src"] < HI) if p == 0 else (R["src"] >= HI)
                caps = np.zeros(gm["n_pairs"], np.int64)
                elists = {}
                for c in range(N_CORES):
                    es = np.where(sel_pass & (ecore == c))[0]
                    pr = epair[es]
                    order = np.argsort(pr, kind="stable")
                    es = es[order]
                    cnt = np.bincount(pr[order], minlength=gm["n_pairs"])
                    caps = np.maximum(caps, cnt)
                    elists[c] = (es, cnt)
                cap_tiles = -(-caps // TILE)
                n_slots = int(cap_tiles.sum()) * TILE
                pairs_sched = [(int(q), int(cap_tiles[q]))
                               for q in range(gm["n_pairs"]) if cap_tiles[q] > 0]
                for c in range(N_CORES):
                    es, cnt = elists[c]
                    idx = np.zeros(n_slots, np.int64)
                    dstl = np.full(n_slots, -1, np.int64)
                    od = np.ones(n_slots, np.int64)
                    idg = np.ones(n_slots, np.int64)
                    off = 0
                    pos = 0
                    for q in range(gm["n_pairs"]):
                        k = int(cnt[q])
                        cap = int(cap_tiles[q]) * TILE
                        e = es[pos:pos + k]
                        pos += k
                        if cap == 0:
                            continue
                        s_ = R["src"][e]
                        d_ = R["dst"][e]
                        idx[off:off + k] = s_ - (HI if p == 1 else 0)
                        dstl[off:off + k] = gm["slot"][d_]
                        od[off:off + k] = R["outdeg"][s_]
                        idg[off:off + k] = R["indeg"][d_]
                        off += cap
                    pc = per_core[c]
                    pc["idx"].append(idx)
                    pc["dstl"].append(dstl)
                    pc["od"].append(od)
                    pc["idg"].append(idg)
                chunks = []
                o = 0
                while o < n_slots:
                    n = min(CHUNK, n_slots - o)
                    chunks.append((slot_base + o, n))
                    o += n
                streams.append(dict(rel=rel, p=p, group=gname, tab=tabname,
                                    base=(HI if p == 1 else 0),
                                    slot_off=slot_base, n_slots=n_slots,
                                    chunks=chunks, pairs=pairs_sched))
                slot_base += n_slots

    S = slot_base
    in_maps = []
    ar = np.arange(S)
    for c in range(N_CORES):
        pc = per_core[c]
        idx = np.concatenate(pc["idx"])
        dstl = np.concatenate(pc["dstl"])
        od = np.concatenate(pc["od"])
        idg = np.concatenate(pc["idg"])
        w = np.zeros((16, S // 16), np.int16)
        w[ar % 16, ar // 16] = idx.astype(np.int16)
        idx_dev = np.tile(w, (8, 1))

        def slotwrap(a, dt):
            o = np.zeros((128, S // 128), dt)
            o[ar % 128, ar // 128] = a.astype(dt)
            return o

        m = {
            "g_idx": idx_dev,
            "g_dstl": slotwrap(dstl, np.int16),
            "g_od": slotwrap(np.minimum(od, 2 ** 30), np.int32),
            "g_idg": slotwrap(np.minimum(idg, 2 ** 30), np.int32),
        }
        for t in ("node_feat", "instance_feat", "svc_feat"):
            m[t] = np.ascontiguousarray(inp[t], dtype=np.float32)
        for rel in rels_all:
            m[f"W_{rel}"] = np.ascontiguousarray(inp[f"W_{rel}"], np.float32)
            m[f"b_{rel}"] = np.ascontiguousarray(
                inp[f"b_{rel}"], np.float32).reshape(D, 1)
        in_maps.append(m)

    out_cols = sum(gmeta[g]["n_pairs"] for g, _, _ in GROUPS) * PAIR
    sched = dict(S=S, streams=streams, out_cols=out_cols,
                 groups=[dict(name=g, n_dst=nd,
                              rels=[r for r, _, _ in rels],
                              n_pairs=gmeta[g]["n_pairs"])
                         for g, nd, rels in GROUPS])
    meta = dict(gmeta=gmeta, sched=sched)
    return in_maps, sched, meta


def build_nc(sched, repeats=1, n_cores=N_CORES):
    nc = bacc.Bacc("TRN2", target_bir_lowering=False, debug=False,
                   num_devices=n_cores, num_swdge_queues=N_QUEUES)
    S = sched["S"]
    dram = {}
    for t, n in (("node_feat", 10000), ("instance_feat", 50000),
                 ("svc_feat", 20000)):
        dram[t] = nc.dram_tensor(t, [n, D], mybir.dt.float32,
                                 kind="ExternalInput").ap()
    for g in sched["groups"]:
        for rel in g["rels"]:
            dram[f"W_{rel}"] = nc.dram_tensor(
                f"W_{rel}", [D, D], mybir.dt.float32, kind="ExternalInput").ap()
            dram[f"b_{rel}"] = nc.dram_tensor(
                f"b_{rel}", [D, 1], mybir.dt.float32, kind="ExternalInput").ap()
    g_idx = nc.dram_tensor("g_idx", [128, S // 16], mybir.dt.int16,
                           kind="ExternalInput").ap()
    g_dstl = nc.dram_tensor("g_dstl", [128, S // 128], mybir.dt.int16,
                            kind="ExternalInput").ap()
    g_od = nc.dram_tensor("g_od", [128, S // 128], mybir.dt.int32,
                          kind="ExternalInput").ap()
    g_idg = nc.dram_tensor("g_idg", [128, S // 128], mybir.dt.int32,
                           kind="ExternalInput").ap()
    o_d = nc.dram_tensor("out", [128, sched["out_cols"]], mybir.dt.float32,
                         kind="ExternalOutput").ap()

    C = S // 128
    with tile.TileContext(nc) as tc:
        with (tc.tile_pool(name="meta", bufs=1) as meta,
              tc.tile_pool(name="stage", bufs=STAGE_BUFS) as stage,
              tc.tile_pool(name="mask", bufs=MASK_BUFS) as maskp,
              tc.tile_pool(name="psA", bufs=PSA_BUFS, space="PSUM") as psA,
              tc.tile_pool(name="psB", bufs=2, space="PSUM") as psB,
              tc.tile_pool(name="evac", bufs=1) as evacp,
              tc.tile_pool(name="outp", bufs=3) as outp):
            # ---- persistent metadata ----
            idx_t = meta.tile([128, S // 16], mybir.dt.int16, tag="idx")
            nc.sync.dma_start(out=idx_t[:], in_=g_idx[:])
            dstl_i = meta.tile([128, C], mybir.dt.int16, tag="dstli")
            nc.sync.dma_start(out=dstl_i[:], in_=g_dstl[:])
            od_t = meta.tile([128, C], mybir.dt.int32, tag="od")
            nc.sync.dma_start(out=od_t[:], in_=g_od[:])
            idg_t = meta.tile([128, C], mybir.dt.int32, tag="idg")
            nc.sync.dma_start(out=idg_t[:], in_=g_idg[:])
            dstl_f = meta.tile([128, C], mybir.dt.bfloat16, tag="dstlf")
            nc.vector.tensor_copy(dstl_f[:], dstl_i[:])
            # s = 1/sqrt(max(od,1)*max(id,1))
            odf = meta.tile([128, C], mybir.dt.float32, tag="odf")
            nc.vector.tensor_scalar(out=odf[:], in0=od_t[:], scalar1=1,
                                    scalar2=None, op0=mybir.AluOpType.max)
            idf = meta.tile([128, C], mybir.dt.float32, tag="idf")
            nc.vector.tensor_scalar(out=idf[:], in0=idg_t[:], scalar1=1,
                                    scalar2=None, op0=mybir.AluOpType.max)
            prod = meta.tile([128, C], mybir.dt.float32, tag="prod")
            nc.vector.tensor_tensor(out=prod[:], in0=odf[:], in1=idf[:],
                                    op=mybir.AluOpType.mult)
            sq = meta.tile([128, C], mybir.dt.float32, tag="sq")
            nc.scalar.sqrt(sq[:], prod[:])
            s_t = meta.tile([128, C], mybir.dt.float32, tag="s")
            nc.vector.reciprocal(s_t[:], sq[:])
            # iota constants
            iota_i = meta.tile([128, PAIR], mybir.dt.int32, tag="iotai")
            nc.gpsimd.iota(iota_i[:], pattern=[[1, PAIR]], base=0,
                           channel_multiplier=0)
            iota_rep = meta.tile([128, MASK_BATCH, PAIR], mybir.dt.bfloat16,
                                 tag="iotarep")
            for _mb in range(MASK_BATCH):
                nc.vector.tensor_copy(iota_rep[:, _mb, :], iota_i[:])
            # weights & biases
            wbf = {}
            bias = {}
            for g in sched["groups"]:
                k = float(len(g["rels"]))
                bts = []
                for rel in g["rels"]:
                    wt = meta.tile([D, D], mybir.dt.float32, tag=f"wf_{rel}")
                    nc.sync.dma_start(out=wt[:], in_=dram[f"W_{rel}"][:])
                    wb = meta.tile([D, D], mybir.dt.bfloat16, tag=f"wb_{rel}")
                    nc.vector.tensor_copy(wb[:], wt[:])
                    wbf[rel] = wb
                    bt = meta.tile([D, 1], mybir.dt.float32, tag=f"bf_{rel}")
                    nc.sync.dma_start(out=bt[:], in_=dram[f"b_{rel}"][:])
                    bts.append(bt)
                bsum = meta.tile([D, 1], mybir.dt.float32, tag=f"bs_{g['name']}")
                nc.vector.tensor_scalar(out=bsum[:], in0=bts[0][:],
                                        scalar1=1.0 / k, scalar2=None,
                                        op0=mybir.AluOpType.mult)
                for bi, bt in enumerate(bts[1:]):
                    btmp = meta.tile([D, 1], mybir.dt.float32,
                                     tag=f"btmp_{g['name']}_{bi}")
                    nc.vector.tensor_scalar(out=btmp[:], in0=bt[:],
                                            scalar1=1.0 / k, scalar2=None,
                                            op0=mybir.AluOpType.mult)
                    nc.vector.tensor_tensor(out=bsum[:], in0=bsum[:],
                                            in1=btmp[:],
                                            op=mybir.AluOpType.add)
                bias[g["name"]] = bsum

            for _rep in range(repeats):
                evacs = {}
                for g in sched["groups"]:
                    for rel in g["rels"]:
                        for q in range(g["n_pairs"]):
                            ev_t = evacp.tile(
                                [128, PAIR], mybir.dt.bfloat16,
                                tag=f"ev_{rel}_{q}")
                            evacs[(rel, q)] = ev_t
                written = set()

                col_off = 0
                group_col = {}
                for g in sched["groups"]:
                    group_col[g["name"]] = col_off
                    col_off += g["n_pairs"] * PAIR

                for g in sched["groups"]:
                    gst = [st for st in sched["streams"]
                           if st["group"] == g["name"]]
                    for st in gst:
                        rel = st["rel"]
                        tiles = []
                        for (q, ntl) in st["pairs"]:
                            for t in range(ntl):
                                tiles.append((q, t, ntl))
                        pending_evacs = []

                        def flush_evac(force=False):
                            while pending_evacs and (force or
                                                     len(pending_evacs) > 3):
                                rel_, q_, ps_ = pending_evacs.pop(0)
                                ev = evacs[(rel_, q_)]
                                if (rel_, q_) in written:
                                    nc.vector.tensor_tensor(
                                        out=ev[:], in0=ev[:], in1=ps_[:],
                                        op=mybir.AluOpType.add)
                                else:
                                    nc.scalar.activation(
                                        ev[:], ps_[:],
                                        mybir.ActivationFunctionType.Copy)
                                    written.add((rel_, q_))

                        ti = 0
                        for (coff, cn) in st["chunks"]:
                            qn = (coff // CHUNK) % N_QUEUES
                            nt = cn // TILE
                            gf32 = stage.tile([128, 8, D], mybir.dt.float32,
                                              tag="stagef")
                            nc.gpsimd.dma_gather(
                                out_ap=gf32[:, :nt, :],
                                in_ap=dram[st["tab"]][st["base"]:, :],
                                idxs_ap=idx_t[:, coff // 16:(coff + cn) // 16],
                                num_idxs=cn, num_idxs_reg=cn, elem_size=D,
                                queue_num=qn)
                            gstg = stage.tile([128, 8, D], mybir.dt.bfloat16,
                                              tag="stageb")
                            c0 = coff // 128
                            # one-hot masks, MASK_BATCH tiles per DVE op
                            mtiles = {}
                            j0 = 0
                            while j0 < nt:
                                mb = min(MASK_BATCH, nt - j0)
                                m4 = maskp.tile([128, MASK_BATCH, PAIR],
                                                mybir.dt.bfloat16, tag="mask")
                                nc.vector.tensor_tensor(
                                    out=m4[:, :mb, :],
                                    in0=iota_rep[:, :mb, :],
                                    in1=dstl_f[:, c0 + j0:c0 + j0 + mb]
                                        .unsqueeze(-1)
                                        .to_broadcast([128, mb, PAIR]),
                                    op=mybir.AluOpType.is_equal)
                                for jj in range(mb):
                                    mtiles[j0 + jj] = m4[:, jj, :]
                                j0 += mb
                            for j in range(nt):
                                q, t, ntl = tiles[ti]
                                col = c0 + j
                                # scaled bf16 cast (split ACT / DVE)
                                if DVE_CAST_MOD and j % DVE_CAST_MOD == 0:
                                    nc.vector.tensor_tensor(
                                        out=gstg[:, j, :], in0=gf32[:, j, :],
                                        in1=s_t[:, col:col + 1]
                                            .to_broadcast([128, D]),
                                        op=mybir.AluOpType.mult)
                                else:
                                    nc.scalar.activation(
                                        gstg[:, j, :], gf32[:, j, :],
                                        mybir.ActivationFunctionType.Copy,
                                        scale=s_t[:, col:col + 1])
                                first = (t == 0)
                                last = (t == ntl - 1)
                                if first:
                                    ps = psA.tile([128, PAIR],
                                                  mybir.dt.float32,
                                                  space="PSUM", tag="agg")
                                    st.setdefault("_ps", {})[q] = ps
                                ps = st["_ps"][q]
                                nc.tensor.matmul(ps[:], lhsT=gstg[:, j, :],
                                                 rhs=mtiles[j], start=first,
                                                 stop=last)
                                if last:
                                    pending_evacs.append((rel, q, ps))
                                    flush_evac()
                                ti += 1
                        flush_evac(force=True)
                        st.pop("_ps", None)
                    # group finalize
                    k = len(g["rels"])
                    for q in range(g["n_pairs"]):
                        todo = [rel for rel in g["rels"] if (rel, q) in written]
                        gcol = group_col[g["name"]] + q * PAIR
                        if not todo:
                            zt = outp.tile([128, PAIR], mybir.dt.float32,
                                           tag="out")
                            nc.vector.memset(zt[:], 0.0)
                            ot = outp.tile([128, PAIR], mybir.dt.float32,
                                           tag="out")
                            nc.scalar.activation(
                                ot[:], zt[:],
                                mybir.ActivationFunctionType.Relu,
                                bias=bias[g["name"]][:], scale=0.0)
                            nc.sync.dma_start(out=o_d[:, gcol:gcol + PAIR],
                                              in_=ot[:])
                            continue
                        ops = psB.tile([128, PAIR], mybir.dt.float32,
                                       space="PSUM", tag="oud")
                        for i, rel in enumerate(todo):
                            nc.tensor.matmul(ops[:], lhsT=wbf[rel][:],
                                             rhs=evacs[(rel, q)][:],
                                             start=(i == 0),
                                             stop=(i == len(todo) - 1))
                        ot = outp.tile([128, PAIR], mybir.dt.float32,
                                       tag="out")
                        nc.scalar.activation(
                            ot[:], ops[:], mybir.ActivationFunctionType.Relu,
                            bias=bias[g["name"]][:], scale=1.0 / k)
                        nc.sync.dma_start(out=o_d[:, gcol:gcol + PAIR],
                                          in_=ot[:])
    nc.compile()
    return nc


def unshard(results, meta):
    gmeta = meta["gmeta"]
    sched = meta["sched"]
    col = 0
    outs = []
    for g in sched["groups"]:
        gm = gmeta[g["name"]]
        full = np.zeros((g["n_dst"], D), np.float32)
        ids = np.arange(g["n_dst"])
        for c in range(N_CORES):
            o = results[c]["out"]
            sel = gm["core"] == c
            cols = col + gm["pair"][sel] * PAIR + gm["slot"][sel]
            full[ids[sel]] = o[:, cols].T
        outs.append(full)
        col += g["n_pairs"] * PAIR
    return np.concatenate(outs, axis=0)


def kernel(**inputs) -> np.ndarray:
    in_maps, sched, meta = prepare(inputs)
    nc = build_nc(sched)
    res = run_bass_kernel_spmd(nc, in_maps, core_ids=list(range(N_CORES)))
    return unshard(res.results, meta)


if __name__ == "__main__":
    rng = np.random.default_rng(0)
    demo = {}
    demo["node_feat"] = rng.standard_normal((10000, D)).astype(np.float32)
    demo["instance_feat"] = rng.standard_normal((50000, D)).astype(np.float32)
    demo["svc_feat"] = rng.standard_normal((20000, D)).astype(np.float32)
    for rel, ns, nd in (("sc", 20000, 20000), ("in", 50000, 10000),
                        ("ni", 10000, 50000), ("ii", 50000, 50000),
                        ("si", 20000, 50000), ("is", 50000, 20000)):
        demo[f"e_{rel}_src"] = rng.integers(0, ns, 200000).astype(np.int64)
        demo[f"e_{rel}_dst"] = rng.integers(0, nd, 200000).astype(np.int64)
        demo[f"W_{rel}"] = (rng.standard_normal((D, D)) * 0.05).astype(np.float32)
        demo[f"b_{rel}"] = (rng.standard_normal(D) * 0.05).astype(np.float32)
    out = kernel(**demo)
    print("kernel output", out.shape, out.dtype, float(np.abs(out).sum()))
